# revision 5
# baseline (speedup 1.0000x reference)
"""Trainium2 Bass kernel for nn_CrossAttentionFusion (V=3, B=8192, H=2048, NH=16).

Strategy:
  - Data-parallel: batch B=8192 split across 8 NeuronCores (Bc=1024 each).
  - Feature-major activations on device: every tensor is [H, Bc] so all
    projections are PE matmuls (lhsT = W^T tile [128h x 128g], moving = act
    [128h x 512b]) with no on-device transposes.  Host transposes views and
    weights once (pure layout, no math).
  - fp32r matmuls (TF32-class rounding, 1 cyc/row at N=512 -> ~238ns/MM).
  - Softmax over V-1=2 key views collapses to a sigmoid:
        a0 = sigmoid((qh . (kh0 - kh1)) / sqrt(HD))
        ctx = v2_1 + a0 * (v2_0 - v2_1)
    so the k-side inner projection only needs Wik @ (k[s0] - k[s1]).
  - 27 HxH matmul-equivalents per core, PE-bound.
"""

import math

import numpy as np

V = 3
B = 8192
H = 2048
NH = 16
HD = H // NH
EPS = 1e-5
N_CORES = 8
BC = B // N_CORES          # 1024 batch columns per core
NT = H // 128              # 16 h-tiles
HALF = 512                 # matmul moving free dim
SCALE = 1.0 / math.sqrt(HD)

# others[i] = sources of keys/values for query view i
S0 = [1, 0, 0]
S1 = [2, 2, 1]

_CACHE = {}


def _build_program():
    import concourse.bacc as bacc
    import concourse.tile as tile
    import concourse.mybir as mybir

    f32 = mybir.dt.float32
    f32r = mybir.dt.float32r
    AF = mybir.ActivationFunctionType
    ALU = mybir.AluOpType

    nc = bacc.Bacc("TRN2", target_bir_lowering=False, debug=False,
                   num_devices=N_CORES)

    # ---- External I/O ----
    xT = nc.dram_tensor("xT", [V, H, BC], f32r, kind="ExternalInput").ap()
    wq = nc.dram_tensor("wq", [V, H, H], f32r, kind="ExternalInput").ap()
    wk = nc.dram_tensor("wk", [V, H, H], f32r, kind="ExternalInput").ap()
    wv = nc.dram_tensor("wv", [V, H, H], f32r, kind="ExternalInput").ap()
    wiq = nc.dram_tensor("wiq", [V, H, H], f32r, kind="ExternalInput").ap()
    wik = nc.dram_tensor("wik", [V, H, H], f32r, kind="ExternalInput").ap()
    wiv = nc.dram_tensor("wiv", [V, H, H], f32r, kind="ExternalInput").ap()
    wo = nc.dram_tensor("wo", [V, H, H], f32r, kind="ExternalInput").ap()
    wout = nc.dram_tensor("wout", [V, H, H], f32r, kind="ExternalInput").ap()
    # bias pack: [21,128,16]: bq(0-2) bk(3-5) bv(6-8) biq(9-11) biv(12-14)
    # bo(15-17) bout(18) gamma(19) beta(20); slice [:, :, gt] is per-partition
    bpk = nc.dram_tensor("bpk", [21, 128, NT], f32, kind="ExternalInput").ap()
    onesc = nc.dram_tensor("onesc", [128, 128], f32r, kind="ExternalInput").ap()
    out = nc.dram_tensor("out", [H, BC], f32, kind="ExternalOutput").ap()

    # ---- DRAM scratch ----
    def scr(name, dt=f32r):
        return nc.dram_tensor(name, [V, H, BC], dt).ap()

    q_s, k_s, v_s = scr("q_s"), scr("k_s"), scr("v_s")
    q2_s, dk2_s = scr("q2_s"), scr("dk2_s")
    dv2_s, v21_s = scr("dv2_s"), scr("v21_s")
    proj_s = scr("proj_s", f32)

    with tile.TileContext(nc) as tc:
        ctxs = []

        def pool(name, bufs):
            p = tc.tile_pool(name=name, bufs=bufs)
            ctxs.append(p)
            return p.__enter__()

        xin = pool("xin", 1)        # 16 tags x 4KB: resident input (64KB/p)
        res2 = pool("res2", 1)      # 16 tags x 4KB: second resident (64KB/p)
        wp = pool("wp", 1)          # 16 tags x 2KB: weight stream (32KB/p)
        stp = pool("stp", 3)        # 1 tag x 3: DRAM streaming (12KB/p)
        tmp = pool("tmp", 2)        # 1 tag x 2: elementwise temps (8KB/p)
        evp = pool("evp", 2)        # 1 tag x 2: eviction staging (4KB/p)
        a0p = pool("a0p", 2)        # 1 tag x 2: [1,512] attn weights (4KB/p)
        cst = pool("cst", 1)        # constants
        psp = tc.tile_pool(name="psp", bufs=1, space="PSUM")
        ctxs.append(psp)
        psp = psp.__enter__()

        # constants
        bias_sb = cst.tile([128, 21, NT], f32)
        nc.sync.dma_start(bias_sb[:], bpk.rearrange("s p f -> p s f"))
        ones_r = cst.tile([128, 1], f32r)
        nc.sync.dma_start(ones_r[:], onesc[:, 0:1])
        ones1_r = cst.tile([1, 128], f32r)
        nc.sync.dma_start(ones1_r[:], onesc[0:1, :])
        ones_f = cst.tile([128, 1], f32)
        nc.vector.memset(ones_f[:], 1.0)
        ones1_f = cst.tile([1, 128], f32)
        nc.vector.memset(ones1_f[:], 1.0)
        eps_t = cst.tile([1, 1], f32)
        nc.vector.memset(eps_t[:], EPS)

        def load16(src2d, pl, dt=f32r, tag="x"):
            ts = []
            for t in range(NT):
                tl = pl.tile([128, BC], dt, tag=f"{tag}{t}")
                nc.sync.dma_start(tl[:], src2d[t * 128:(t + 1) * 128, :])
                ts.append(tl)
            return ts

        def evict(dst_ap, ps_ap, bidx, gt, dt):
            """PSUM -> SBUF/out with optional bias add (fused into ACT)."""
            if bidx is None:
                nc.scalar.activation(dst_ap, ps_ap, AF.Copy)
            else:
                nc.scalar.activation(dst_ap, ps_ap, AF.Identity,
                                     bias=bias_sb[:, bidx, gt:gt + 1])

        def proj(w2d, xt, dst, bidx=None, dt=f32r, dst_sb=None):
            """dst[g,b] = sum_h w2d[h,g] * x[h,b] (+bias[g]).

            w2d: DRAM [H,H] (h-major).  xt: 16 resident [128,BC] tiles.
            dst: DRAM [H,BC] ap (if dst_sb None) else list of 16 SBUF tiles.
            """
            for gg in range(4):
                wb = []
                for ht in range(NT):
                    w = wp.tile([128, 512], f32r, tag=f"w{ht}")
                    nc.sync.dma_start(
                        w[:], w2d[ht * 128:(ht + 1) * 128,
                                  gg * 512:(gg + 1) * 512])
                    wb.append(w)
                for hf in range(2):
                    pts = [psp.tile([128, 512], f32, tag=f"pp{gi + 4 * hf}",
                                    name=f"pt{gi}") for gi in range(4)]
                    for ht in range(NT):
                        for gi in range(4):
                            nc.tensor.matmul(
                                pts[gi][:],
                                wb[ht][:, gi * 128:(gi + 1) * 128],
                                xt[ht][:, hf * HALF:(hf + 1) * HALF],
                                start=(ht == 0), stop=(ht == NT - 1))
                    for gi in range(4):
                        gt = gg * 4 + gi
                        if dst_sb is not None:
                            evict(dst_sb[gt][:, hf * HALF:(hf + 1) * HALF],
                                  pts[gi][:], bidx, gt, dt)
                        else:
                            et = evp.tile([128, 512], dt, tag="ev")
                            evict(et[:], pts[gi][:], bidx, gt, dt)
                            nc.sync.dma_start(
                                dst[gt * 128:(gt + 1) * 128,
                                    hf * HALF:(hf + 1) * HALF], et[:])

        def proj2(w2d, xa, xb, dsta, dstb, bidx_b):
            """Two outputs sharing one weight stream (dv2 & v21)."""
            for gg in range(8):
                wb = []
                for ht in range(NT):
                    w = wp.tile([128, 256], f32r, tag=f"w{ht}")
                    nc.sync.dma_start(
                        w[:], w2d[ht * 128:(ht + 1) * 128,
                                  gg * 256:(gg + 1) * 256])
                    wb.append(w)
                for hf in range(2):
                    pa = [psp.tile([128, 512], f32, tag=f"pp{gi + 4 * hf}",
                                   name=f"pa{gi}") for gi in range(2)]
                    pb = [psp.tile([128, 512], f32, tag=f"pp{gi + 2 + 4 * hf}",
                                   name=f"pb{gi}") for gi in range(2)]
                    for ht in range(NT):
                        for gi in range(2):
                            ws = wb[ht][:, gi * 128:(gi + 1) * 128]
                            nc.tensor.matmul(
                                pa[gi][:], ws,
                                xa[ht][:, hf * HALF:(hf + 1) * HALF],
                                start=(ht == 0), stop=(ht == NT - 1))
                            nc.tensor.matmul(
                                pb[gi][:], ws,
                                xb[ht][:, hf * HALF:(hf + 1) * HALF],
                                start=(ht == 0), stop=(ht == NT - 1))
                    for gi in range(2):
                        gt = gg * 2 + gi
                        ea = evp.tile([128, 512], f32r, tag="ev")
                        evict(ea[:], pa[gi][:], None, gt, f32r)
                        nc.sync.dma_start(
                            dsta[gt * 128:(gt + 1) * 128,
                                 hf * HALF:(hf + 1) * HALF], ea[:])
                        eb = evp.tile([128, 512], f32r, tag="ev")
                        evict(eb[:], pb[gi][:], bidx_b, gt, f32r)
                        nc.sync.dma_start(
                            dstb[gt * 128:(gt + 1) * 128,
                                 hf * HALF:(hf + 1) * HALF], eb[:])

        # ================= P1: q/k/v projections =================
        for v in range(V):
            xt = load16(xT[v], xin, tag="x")
            proj(wk[v], xt, k_s[v], bidx=3 + v)
            proj(wv[v], xt, v_s[v], bidx=6 + v)
            proj(wq[v], xt, q_s[v], bidx=0 + v)

        # ================= P2: q2 = Wiq @ q =================
        for i in range(V):
            qt = load16(q_s[i], xin, tag="x")
            proj(wiq[i], qt, q2_s[i], bidx=9 + i)

        # ================= P3: dk2 = Wik @ (k[s0]-k[s1]) =================
        for i in range(V):
            kd = []
            for t in range(NT):
                k0 = stp.tile([128, BC], f32r, tag="st")
                k1 = stp.tile([128, BC], f32r, tag="st")
                nc.sync.dma_start(k0[:], k_s[S0[i]][t * 128:(t + 1) * 128, :])
                nc.sync.dma_start(k1[:], k_s[S1[i]][t * 128:(t + 1) * 128, :])
                kt = xin.tile([128, BC], f32r, tag=f"x{t}")
                nc.vector.tensor_tensor(kt[:], k0[:], k1[:], ALU.subtract)
                kd.append(kt)
            proj(wik[i], kd, dk2_s[i])

        # ===== P4: dv2 = Wiv @ (v[s0]-v[s1]);  v21 = Wiv @ v[s1] + biv =====
        for i in range(V):
            vd, v1 = [], []
            for t in range(NT):
                v1t = res2.tile([128, BC], f32r, tag=f"r{t}")
                nc.sync.dma_start(v1t[:], v_s[S1[i]][t * 128:(t + 1) * 128, :])
                v0 = stp.tile([128, BC], f32r, tag="st")
                nc.sync.dma_start(v0[:], v_s[S0[i]][t * 128:(t + 1) * 128, :])
                vdt = xin.tile([128, BC], f32r, tag=f"x{t}")
                nc.vector.tensor_tensor(vdt[:], v0[:], v1t[:], ALU.subtract)
                vd.append(vdt)
                v1.append(v1t)
            proj2(wiv[i], vd, v1, dv2_s[i], v21_s[i], bidx_b=12 + i)

        # ================= P5: attention + Wo + Wout =================
        for i in range(V):
            # --- A: a0 + ctx, head tile by head tile ---
            ctx_t = []
            for t in range(NT):
                q2t = stp.tile([128, BC], f32r, tag="st")
                dkt = stp.tile([128, BC], f32r, tag="st")
                nc.sync.dma_start(q2t[:], q2_s[i][t * 128:(t + 1) * 128, :])
                nc.sync.dma_start(dkt[:], dk2_s[i][t * 128:(t + 1) * 128, :])
                pt = tmp.tile([128, BC], f32r, tag="tm")
                nc.vector.tensor_tensor(pt[:], q2t[:], dkt[:], ALU.mult)
                dvt = stp.tile([128, BC], f32r, tag="st")
                v1t = stp.tile([128, BC], f32r, tag="st")
                nc.sync.dma_start(dvt[:], dv2_s[i][t * 128:(t + 1) * 128, :])
                nc.sync.dma_start(v1t[:], v21_s[i][t * 128:(t + 1) * 128, :])
                t2 = tmp.tile([128, BC], f32r, tag="tm")
                for hf in range(2):
                    sl = slice(hf * HALF, (hf + 1) * HALF)
                    cs = psp.tile([128, 512], f32, tag=f"pp{(2 * t + hf) % 8}")
                    nc.tensor.matmul(cs[0:1, :], ones_r[:], pt[:, sl],
                                     start=True, stop=True)
                    a0t = a0p.tile([1, 512], f32r, tag="a0")
                    nc.scalar.activation(a0t[:], cs[0:1, :], AF.Sigmoid,
                                         scale=SCALE)
                    bc = psp.tile([128, 512], f32,
                                  tag=f"pp{(2 * t + hf + 4) % 8}")
                    nc.tensor.matmul(bc[:], ones1_r[:], a0t[:],
                                     start=True, stop=True)
                    nc.vector.tensor_tensor(t2[:, sl], dvt[:, sl], bc[:],
                                            ALU.mult)
                ct = xin.tile([128, BC], f32r, tag=f"x{t}")
                nc.vector.tensor_tensor(ct[:], t2[:], v1t[:], ALU.add)
                ctx_t.append(ct)
            # --- B: att = Wo[i] @ ctx + bo ---
            att = [res2.tile([128, BC], f32r, tag=f"r{t}", name=f"att{t}")
                   for t in range(NT)]
            proj(wo[i], ctx_t, None, bidx=15 + i, dst_sb=att)
            # --- C: proj_s[i] = Wout_i @ att (+bout once) ---
            proj(wout[i], att, proj_s[i], bidx=(18 if i == 0 else None),
                 dt=f32)

        # ================= P6: residual + LayerNorm =================
        xln = []
        sx = [psp.tile([128, 512], f32, tag="pp4", name="sx0"),
              psp.tile([128, 512], f32, tag="pp5", name="sx1")]
        sxx = [psp.tile([128, 512], f32, tag="pp6", name="sxx0"),
               psp.tile([128, 512], f32, tag="pp7", name="sxx1")]
        for t in range(NT):
            r0 = stp.tile([128, BC], f32, tag="st")
            nc.sync.dma_start(r0[:], xT[0].bitcast(f32)[t * 128:(t + 1) * 128, :])
            p0 = stp.tile([128, BC], f32, tag="st")
            nc.sync.dma_start(p0[:], proj_s[0][t * 128:(t + 1) * 128, :])
            p1 = stp.tile([128, BC], f32, tag="st")
            nc.sync.dma_start(p1[:], proj_s[1][t * 128:(t + 1) * 128, :])
            p2 = stp.tile([128, BC], f32, tag="st")
            nc.sync.dma_start(p2[:], proj_s[2][t * 128:(t + 1) * 128, :])
            xt = xin.tile([128, BC], f32, tag=f"x{t}")
            nc.vector.tensor_tensor(xt[:], r0[:], p0[:], ALU.add)
            nc.vector.tensor_tensor(xt[:], xt[:], p1[:], ALU.add)
            nc.vector.tensor_tensor(xt[:], xt[:], p2[:], ALU.add)
            sq = tmp.tile([128, BC], f32, tag="tm")
            nc.vector.tensor_tensor(sq[:], xt[:], xt[:], ALU.mult)
            for hf in range(2):
                sl = slice(hf * HALF, (hf + 1) * HALF)
                nc.tensor.matmul(sx[hf][0:1, :], ones_f[:], xt[:, sl],
                                 start=(t == 0), stop=(t == NT - 1))
                nc.tensor.matmul(sxx[hf][0:1, :], ones_f[:], sq[:, sl],
                                 start=(t == 0), stop=(t == NT - 1))
            xln.append(xt)
        # stats: mu, var, rstd, Bv = -mu*rstd  (all [1, BC])
        mu = res2.tile([1, BC], f32, tag="ln0")
        m2 = res2.tile([1, BC], f32, tag="ln1")
        for hf in range(2):
            sl = slice(hf * HALF, (hf + 1) * HALF)
            nc.scalar.activation(mu[:, sl], sx[hf][0:1, :], AF.Copy,
                                 scale=1.0 / H)
            nc.scalar.activation(m2[:, sl], sxx[hf][0:1, :], AF.Copy,
                                 scale=1.0 / H)
            msq = a0p.tile([1, 512], f32, tag="a0")
            nc.vector.tensor_tensor(msq[:], mu[:, sl], mu[:, sl], ALU.mult)
            nc.vector.tensor_tensor(m2[:, sl], m2[:, sl], msq[:],
                                    ALU.subtract)
        nc.scalar.activation(m2[:], m2[:], AF.Sqrt, bias=eps_t[:])
        nc.vector.reciprocal(m2[:], m2[:])          # m2 = rstd
        nc.vector.tensor_tensor(mu[:], mu[:], m2[:], ALU.mult)
        nc.scalar.activation(mu[:], mu[:], AF.Copy, scale=-1.0)  # mu = -mu*rstd
        # broadcast rstd/Bv to [128, BC] via K=1 fp32 matmuls
        A_sb = res2.tile([128, BC], f32, tag="r0")
        B_sb = res2.tile([128, BC], f32, tag="r1")
        for hf in range(2):
            sl = slice(hf * HALF, (hf + 1) * HALF)
            pa = psp.tile([128, 512], f32, tag="pp0")
            nc.tensor.matmul(pa[:], ones1_f[:], m2[:, sl], start=True,
                             stop=True)
            nc.scalar.activation(A_sb[:, sl], pa[:], AF.Copy)
            pb = psp.tile([128, 512], f32, tag="pp1")
            nc.tensor.matmul(pb[:], ones1_f[:], mu[:, sl], start=True,
                             stop=True)
            nc.scalar.activation(B_sb[:, sl], pb[:], AF.Copy)
        for t in range(NT):
            n1 = tmp.tile([128, BC], f32, tag="tm")
            nc.vector.tensor_tensor(n1[:], xln[t][:], A_sb[:], ALU.mult)
            nc.vector.tensor_tensor(n1[:], n1[:], B_sb[:], ALU.add)
            nc.vector.tensor_scalar(
                out=n1[:], in0=n1[:],
                scalar1=bias_sb[:, 19, t:t + 1], scalar2=bias_sb[:, 20, t:t + 1],
                op0=ALU.mult, op1=ALU.add)
            nc.sync.dma_start(out[t * 128:(t + 1) * 128, :], n1[:])

        for p in reversed(ctxs):
            p.__exit__(None, None, None)

    nc.compile()
    return nc


def _prep_host(inputs):
    """Transpose/pack host inputs (layout only, no math)."""
    views = np.asarray(inputs["views"], np.float32)

    def t3(a):
        return np.ascontiguousarray(np.asarray(a, np.float32).transpose(0, 2, 1))

    w = {
        "wq": t3(inputs["Wq"]), "wk": t3(inputs["Wk"]), "wv": t3(inputs["Wv"]),
        "wiq": t3(inputs["Wiq"]), "wik": t3(inputs["Wik"]),
        "wiv": t3(inputs["Wiv"]), "wo": t3(inputs["Wo"]),
        "wout": np.ascontiguousarray(
            np.asarray(inputs["Wout"], np.float32).T.reshape(V, H, H)),
    }

    def bcol(vec):
        return np.asarray(vec, np.float32).reshape(NT, 128).T

    bp = np.zeros((21, 128, NT), np.float32)
    for v in range(V):
        bp[0 + v] = bcol(inputs["bq"][v])
        bp[3 + v] = bcol(inputs["bk"][v])
        bp[6 + v] = bcol(inputs["bv"][v])
        bp[9 + v] = bcol(inputs["biq"][v])
        bp[12 + v] = bcol(inputs["biv"][v])
        bp[15 + v] = bcol(inputs["bo"][v])
    bp[18] = bcol(inputs["bout"])
    bp[19] = bcol(inputs["gamma"])
    bp[20] = bcol(inputs["beta"])
    w["bpk"] = bp
    w["onesc"] = np.ones((128, 128), np.float32)

    xts = []
    for c in range(N_CORES):
        sl = views[:, c * BC:(c + 1) * BC, :]
        xts.append(np.ascontiguousarray(sl.transpose(0, 2, 1)))
    return w, xts


def kernel(**inputs):
    from concourse.bass_utils import run_bass_kernel_spmd

    trace = bool(_CACHE.get("trace", False))
    if "nc" not in _CACHE:
        _CACHE["nc"] = _build_program()
    nc = _CACHE["nc"]

    w, xts = _prep_host(inputs)
    in_maps = []
    for c in range(N_CORES):
        m = dict(w)
        m["xT"] = xts[c]
        in_maps.append(m)

    res = run_bass_kernel_spmd(nc, in_maps, core_ids=list(range(N_CORES)),
                               trace=trace)
    _CACHE["last_result"] = res

    outp = np.empty((B, H), np.float32)
    for c in range(N_CORES):
        outp[c * BC:(c + 1) * BC, :] = res.results[c]["out"].T
    return outp


# revision 8
# speedup vs baseline: 1.0439x; 1.0439x over previous
"""Trainium2 Bass kernel for nn_CrossAttentionFusion (V=3, B=8192, H=2048, NH=16).

Strategy:
  - Data-parallel: batch B=8192 split across 8 NeuronCores (Bc=1024 each).
  - Feature-major activations on device: every tensor is [H, Bc] so all
    projections are PE matmuls (lhsT = W^T tile [128h x 128g], moving = act
    [128h x 512b]) with no on-device transposes.  Host transposes views and
    weights once (pure layout, no math).
  - fp32r matmuls (TF32-class rounding, 1 cyc/row at N=512 -> ~238ns/MM).
  - Softmax over V-1=2 key views collapses to a sigmoid:
        a0 = sigmoid((qh . (kh0 - kh1)) / sqrt(HD))
        ctx = v2_1 + a0 * (v2_0 - v2_1)
    so the k-side inner projection only needs Wik @ (k[s0] - k[s1]).
  - 27 HxH matmul-equivalents per core, PE-bound.
"""

import math

import numpy as np

V = 3
B = 8192
H = 2048
NH = 16
HD = H // NH
EPS = 1e-5
N_CORES = 8
BC = B // N_CORES          # 1024 batch columns per core
NT = H // 128              # 16 h-tiles
HALF = 512                 # matmul moving free dim
SCALE = 1.0 / math.sqrt(HD)

# others[i] = sources of keys/values for query view i
S0 = [1, 0, 0]
S1 = [2, 2, 1]

_CACHE = {}


def _build_program():
    import concourse.bacc as bacc
    import concourse.tile as tile
    import concourse.mybir as mybir

    f32 = mybir.dt.float32
    f32r = mybir.dt.float32r
    AF = mybir.ActivationFunctionType
    ALU = mybir.AluOpType

    nc = bacc.Bacc("TRN2", target_bir_lowering=False, debug=False,
                   num_devices=N_CORES)

    # ---- External I/O ----
    xT = nc.dram_tensor("xT", [V, H, BC], f32r, kind="ExternalInput").ap()
    wq = nc.dram_tensor("wq", [V, H, H], f32r, kind="ExternalInput").ap()
    wk = nc.dram_tensor("wk", [V, H, H], f32r, kind="ExternalInput").ap()
    wv = nc.dram_tensor("wv", [V, H, H], f32r, kind="ExternalInput").ap()
    wiq = nc.dram_tensor("wiq", [V, H, H], f32r, kind="ExternalInput").ap()
    wik = nc.dram_tensor("wik", [V, H, H], f32r, kind="ExternalInput").ap()
    wiv = nc.dram_tensor("wiv", [V, H, H], f32r, kind="ExternalInput").ap()
    wo = nc.dram_tensor("wo", [V, H, H], f32r, kind="ExternalInput").ap()
    wout = nc.dram_tensor("wout", [V, H, H], f32r, kind="ExternalInput").ap()
    # bias pack: [21,128,16]: bq(0-2) bk(3-5) bv(6-8) biq(9-11) biv(12-14)
    # bo(15-17) bout(18) gamma(19) beta(20); slice [:, :, gt] is per-partition
    bpk = nc.dram_tensor("bpk", [21, 128, NT], f32, kind="ExternalInput").ap()
    onesc = nc.dram_tensor("onesc", [128, 128], f32r, kind="ExternalInput").ap()
    out = nc.dram_tensor("out", [H, BC], f32, kind="ExternalOutput").ap()

    # ---- DRAM scratch ----
    def scr(name, dt=f32r):
        return nc.dram_tensor(name, [V, H, BC], dt).ap()

    k_s, v_s = scr("k_s"), scr("v_s")
    q2_s, dk2_s = scr("q2_s"), scr("dk2_s")
    dv2_s, v21_s = scr("dv2_s"), scr("v21_s")
    proj_s = scr("proj_s")

    with tile.TileContext(nc) as tc:
        ctxs = []

        def pool(name, bufs):
            p = tc.tile_pool(name=name, bufs=bufs)
            ctxs.append(p)
            return p.__enter__()

        xin = pool("xin", 1)        # 16 tags x 4KB: resident input (64KB/p)
        res2 = pool("res2", 1)      # 16 tags x 4KB: second resident (64KB/p)
        wp = pool("wp", 1)          # 16 tags x 2KB: weight stream (32KB/p)
        stp = pool("stp", 4)        # 1 tag x 4: DRAM streaming (16KB/p)
        tmp = pool("tmp", 2)        # 1 tag x 2: elementwise temps (8KB/p)
        evp = pool("evp", 2)        # 1 tag x 2: eviction staging (4KB/p)
        a0p = pool("a0p", 3)        # 1 tag x 3: [1,512] attn weights (6KB/p)
        cst = pool("cst", 1)        # constants
        psp = tc.tile_pool(name="psp", bufs=1, space="PSUM")
        ctxs.append(psp)
        psp = psp.__enter__()

        # constants
        bias_sb = cst.tile([128, 21, NT], f32)
        nc.sync.dma_start(bias_sb[:], bpk.rearrange("s p f -> p s f"))
        ones_r = cst.tile([128, 1], f32r)
        nc.sync.dma_start(ones_r[:], onesc[:, 0:1])
        ones1_r = cst.tile([1, 128], f32r)
        nc.sync.dma_start(ones1_r[:], onesc[0:1, :])
        ones_f = cst.tile([128, 1], f32)
        nc.vector.memset(ones_f[:], 1.0)
        ones1_f = cst.tile([1, 128], f32)
        nc.vector.memset(ones1_f[:], 1.0)
        eps_t = cst.tile([1, 1], f32)
        nc.vector.memset(eps_t[:], EPS)

        def load16(src2d, pl, dt=f32r, tag="x"):
            ts = []
            for t in range(NT):
                tl = pl.tile([128, BC], dt, tag=f"{tag}{t}")
                nc.sync.dma_start(tl[:], src2d[t * 128:(t + 1) * 128, :])
                ts.append(tl)
            return ts

        def evict(dst_ap, ps_ap, bidx, gt, dt):
            """PSUM -> SBUF/out with optional bias add (fused into ACT)."""
            if bidx is None:
                nc.scalar.activation(dst_ap, ps_ap, AF.Copy)
            else:
                nc.scalar.activation(dst_ap, ps_ap, AF.Identity,
                                     bias=bias_sb[:, bidx, gt:gt + 1])

        def proj(w2d, xt, dst, bidx=None, dt=f32r, dst_sb=None):
            """dst[g,b] = sum_h w2d[h,g] * x[h,b] (+bias[g]).

            w2d: DRAM [H,H] (h-major).  xt: 16 resident [128,BC] tiles.
            dst: DRAM [H,BC] ap (if dst_sb None) else list of 16 SBUF tiles.
            """
            for gg in range(4):
                wb = []
                for ht in range(NT):
                    w = wp.tile([128, 512], f32r, tag=f"w{ht}")
                    nc.sync.dma_start(
                        w[:], w2d[ht * 128:(ht + 1) * 128,
                                  gg * 512:(gg + 1) * 512])
                    wb.append(w)
                for hf in range(2):
                    pts = [psp.tile([128, 512], f32, tag=f"pp{gi + 4 * hf}",
                                    name=f"pt{gi}") for gi in range(4)]
                    for ht in range(NT):
                        for gi in range(4):
                            nc.tensor.matmul(
                                pts[gi][:],
                                wb[ht][:, gi * 128:(gi + 1) * 128],
                                xt[ht][:, hf * HALF:(hf + 1) * HALF],
                                start=(ht == 0), stop=(ht == NT - 1))
                    for gi in range(4):
                        gt = gg * 4 + gi
                        if dst_sb is not None:
                            evict(dst_sb[gt][:, hf * HALF:(hf + 1) * HALF],
                                  pts[gi][:], bidx, gt, dt)
                        else:
                            et = evp.tile([128, 512], dt, tag="ev")
                            evict(et[:], pts[gi][:], bidx, gt, dt)
                            nc.sync.dma_start(
                                dst[gt * 128:(gt + 1) * 128,
                                    hf * HALF:(hf + 1) * HALF], et[:])

        def proj2(w2d, xa, xb, dsta, dstb, bidx_b):
            """Two outputs sharing one weight stream (dv2 & v21)."""
            for gg in range(8):
                wb = []
                for ht in range(NT):
                    w = wp.tile([128, 256], f32r, tag=f"w{ht}")
                    nc.sync.dma_start(
                        w[:], w2d[ht * 128:(ht + 1) * 128,
                                  gg * 256:(gg + 1) * 256])
                    wb.append(w)
                for hf in range(2):
                    pa = [psp.tile([128, 512], f32, tag=f"pp{gi + 4 * hf}",
                                   name=f"pa{gi}") for gi in range(2)]
                    pb = [psp.tile([128, 512], f32, tag=f"pp{gi + 2 + 4 * hf}",
                                   name=f"pb{gi}") for gi in range(2)]
                    for ht in range(NT):
                        for gi in range(2):
                            ws = wb[ht][:, gi * 128:(gi + 1) * 128]
                            nc.tensor.matmul(
                                pa[gi][:], ws,
                                xa[ht][:, hf * HALF:(hf + 1) * HALF],
                                start=(ht == 0), stop=(ht == NT - 1))
                            nc.tensor.matmul(
                                pb[gi][:], ws,
                                xb[ht][:, hf * HALF:(hf + 1) * HALF],
                                start=(ht == 0), stop=(ht == NT - 1))
                    for gi in range(2):
                        gt = gg * 2 + gi
                        ea = evp.tile([128, 512], f32r, tag="ev")
                        evict(ea[:], pa[gi][:], None, gt, f32r)
                        nc.sync.dma_start(
                            dsta[gt * 128:(gt + 1) * 128,
                                 hf * HALF:(hf + 1) * HALF], ea[:])
                        eb = evp.tile([128, 512], f32r, tag="ev")
                        evict(eb[:], pb[gi][:], bidx_b, gt, f32r)
                        nc.sync.dma_start(
                            dstb[gt * 128:(gt + 1) * 128,
                                 hf * HALF:(hf + 1) * HALF], eb[:])

        # ========= P1: k/v/q projections + q2 folded in (q stays in SBUF) ==
        for v in range(V):
            xt = load16(xT[v], xin, tag="x")
            proj(wk[v], xt, k_s[v], bidx=3 + v)
            proj(wv[v], xt, v_s[v], bidx=6 + v)
            qres = [res2.tile([128, BC], f32r, tag=f"r{t}", name=f"q{t}")
                    for t in range(NT)]
            proj(wq[v], xt, None, bidx=0 + v, dst_sb=qres)
            proj(wiq[v], qres, q2_s[v], bidx=9 + v)

        # ================= P3: dk2 = Wik @ (k[s0]-k[s1]) =================
        for i in range(V):
            kd = []
            for t in range(NT):
                k0 = stp.tile([128, BC], f32r, tag="st")
                k1 = stp.tile([128, BC], f32r, tag="st")
                nc.sync.dma_start(k0[:], k_s[S0[i]][t * 128:(t + 1) * 128, :])
                nc.sync.dma_start(k1[:], k_s[S1[i]][t * 128:(t + 1) * 128, :])
                kt = xin.tile([128, BC], f32r, tag=f"x{t}")
                eng = nc.vector if t % 2 == 0 else nc.gpsimd
                eng.tensor_tensor(kt[:], k0[:], k1[:], ALU.subtract)
                kd.append(kt)
            proj(wik[i], kd, dk2_s[i])

        # ===== P4: dv2 = Wiv @ (v[s0]-v[s1]);  v21 = Wiv @ v[s1] + biv =====
        for i in range(V):
            vd, v1 = [], []
            for t in range(NT):
                v1t = res2.tile([128, BC], f32r, tag=f"r{t}")
                nc.sync.dma_start(v1t[:], v_s[S1[i]][t * 128:(t + 1) * 128, :])
                v0 = stp.tile([128, BC], f32r, tag="st")
                nc.sync.dma_start(v0[:], v_s[S0[i]][t * 128:(t + 1) * 128, :])
                vdt = xin.tile([128, BC], f32r, tag=f"x{t}")
                eng = nc.vector if t % 2 == 0 else nc.gpsimd
                eng.tensor_tensor(vdt[:], v0[:], v1t[:], ALU.subtract)
                vd.append(vdt)
                v1.append(v1t)
            proj2(wiv[i], vd, v1, dv2_s[i], v21_s[i], bidx_b=12 + i)

        # ================= P5: attention + Wo + Wout =================
        # Emission order B(0), A(1), C(0), B(1), A(2), C(1), B(2), C(2):
        # the serial attention chain A(i) (DVE/ACT/tiny-MM latency-bound)
        # hides under the big Wout projection C(i-1).
        def attn_ctx(i):
            ctx_t = []
            for t in range(NT):
                q2t = stp.tile([128, BC], f32r, tag="st", name="q2t")
                dkt = stp.tile([128, BC], f32r, tag="st", name="dkt")
                nc.sync.dma_start(q2t[:], q2_s[i][t * 128:(t + 1) * 128, :])
                nc.sync.dma_start(dkt[:], dk2_s[i][t * 128:(t + 1) * 128, :])
                pt = tmp.tile([128, BC], f32r, tag="tm", name="pt")
                nc.vector.tensor_tensor(pt[:], q2t[:], dkt[:], ALU.mult)
                dvt = stp.tile([128, BC], f32r, tag="st", name="dvt")
                v1t = stp.tile([128, BC], f32r, tag="st", name="v1t")
                nc.sync.dma_start(dvt[:], dv2_s[i][t * 128:(t + 1) * 128, :])
                nc.sync.dma_start(v1t[:], v21_s[i][t * 128:(t + 1) * 128, :])
                t2 = tmp.tile([128, BC], f32r, tag="tm", name="t2")
                for hf in range(2):
                    sl = slice(hf * HALF, (hf + 1) * HALF)
                    cs = psp.tile([128, 512], f32, tag=f"pp{(2 * t + hf) % 8}",
                                  name="cs")
                    nc.tensor.matmul(cs[0:1, :], ones_r[:], pt[:, sl],
                                     start=True, stop=True)
                    a0t = a0p.tile([1, 512], f32r, tag="a0", name="a0t")
                    nc.scalar.activation(a0t[:], cs[0:1, :], AF.Sigmoid,
                                         scale=SCALE)
                    bc = psp.tile([128, 512], f32,
                                  tag=f"pp{(2 * t + hf + 4) % 8}", name="bc")
                    nc.tensor.matmul(bc[:], ones1_r[:], a0t[:],
                                     start=True, stop=True)
                    nc.vector.tensor_tensor(t2[:, sl], dvt[:, sl], bc[:],
                                            ALU.mult)
                ct = xin.tile([128, BC], f32r, tag=f"x{t}", name=f"ctx{t}")
                nc.gpsimd.tensor_tensor(ct[:], t2[:], v1t[:], ALU.add)
                ctx_t.append(ct)
            return ctx_t

        def proj_B(i, ctx_t):
            att = [res2.tile([128, BC], f32r, tag=f"r{t}", name=f"att{t}")
                   for t in range(NT)]
            proj(wo[i], ctx_t, None, bidx=15 + i, dst_sb=att)
            return att

        def proj_C(i, att):
            proj(wout[i], att, proj_s[i], bidx=(18 if i == 0 else None))

        ctx0 = attn_ctx(0)
        att_prev = proj_B(0, ctx0)
        for i in range(1, V):
            ctx_i = attn_ctx(i)
            proj_C(i - 1, att_prev)
            att_prev = proj_B(i, ctx_i)
        proj_C(V - 1, att_prev)

        # ================= P6: residual + LayerNorm =================
        xln = []
        sx = [psp.tile([128, 512], f32, tag="pp4", name="sx0"),
              psp.tile([128, 512], f32, tag="pp5", name="sx1")]
        sxx = [psp.tile([128, 512], f32, tag="pp6", name="sxx0"),
               psp.tile([128, 512], f32, tag="pp7", name="sxx1")]
        for t in range(NT):
            eng = nc.vector if t % 2 == 0 else nc.gpsimd
            en2 = nc.gpsimd if t % 2 == 0 else nc.vector
            r0 = stp.tile([128, BC], f32r, tag="st", name="r0")
            nc.sync.dma_start(r0[:], xT[0][t * 128:(t + 1) * 128, :])
            p0 = stp.tile([128, BC], f32r, tag="st", name="p0")
            nc.sync.dma_start(p0[:], proj_s[0][t * 128:(t + 1) * 128, :])
            p1 = stp.tile([128, BC], f32r, tag="st", name="p1")
            nc.sync.dma_start(p1[:], proj_s[1][t * 128:(t + 1) * 128, :])
            p2 = stp.tile([128, BC], f32r, tag="st", name="p2")
            nc.sync.dma_start(p2[:], proj_s[2][t * 128:(t + 1) * 128, :])
            xt = xin.tile([128, BC], f32r, tag=f"x{t}")
            eng.tensor_tensor(xt[:], r0[:], p0[:], ALU.add)
            eng.tensor_tensor(xt[:], xt[:], p1[:], ALU.add)
            eng.tensor_tensor(xt[:], xt[:], p2[:], ALU.add)
            sq = tmp.tile([128, BC], f32r, tag="tm", name="sq")
            en2.tensor_tensor(sq[:], xt[:], xt[:], ALU.mult)
            for hf in range(2):
                sl = slice(hf * HALF, (hf + 1) * HALF)
                nc.tensor.matmul(sx[hf][0:1, :], ones_r[:], xt[:, sl],
                                 start=(t == 0), stop=(t == NT - 1))
                nc.tensor.matmul(sxx[hf][0:1, :], ones_r[:], sq[:, sl],
                                 start=(t == 0), stop=(t == NT - 1))
            xln.append(xt)
        # stats: mu, var, rstd, Bv = -mu*rstd  (all [1, BC])
        mu = res2.tile([1, BC], f32, tag="ln0")
        m2 = res2.tile([1, BC], f32, tag="ln1")
        for hf in range(2):
            sl = slice(hf * HALF, (hf + 1) * HALF)
            nc.scalar.activation(mu[:, sl], sx[hf][0:1, :], AF.Copy,
                                 scale=1.0 / H)
            nc.scalar.activation(m2[:, sl], sxx[hf][0:1, :], AF.Copy,
                                 scale=1.0 / H)
            msq = a0p.tile([1, 512], f32, tag="a0")
            nc.vector.tensor_tensor(msq[:], mu[:, sl], mu[:, sl], ALU.mult)
            nc.vector.tensor_tensor(m2[:, sl], m2[:, sl], msq[:],
                                    ALU.subtract)
        nc.scalar.activation(m2[:], m2[:], AF.Sqrt, bias=eps_t[:])
        nc.vector.reciprocal(m2[:], m2[:])          # m2 = rstd
        nc.vector.tensor_tensor(mu[:], mu[:], m2[:], ALU.mult)
        nc.scalar.activation(mu[:], mu[:], AF.Copy, scale=-1.0)  # mu = -mu*rstd
        # broadcast rstd/Bv to [128, BC] via K=1 fp32 matmuls
        A_sb = res2.tile([128, BC], f32, tag="r0")
        B_sb = res2.tile([128, BC], f32, tag="r1")
        for hf in range(2):
            sl = slice(hf * HALF, (hf + 1) * HALF)
            pa = psp.tile([128, 512], f32, tag="pp0")
            nc.tensor.matmul(pa[:], ones1_f[:], m2[:, sl], start=True,
                             stop=True)
            nc.scalar.activation(A_sb[:, sl], pa[:], AF.Copy)
            pb = psp.tile([128, 512], f32, tag="pp1")
            nc.tensor.matmul(pb[:], ones1_f[:], mu[:, sl], start=True,
                             stop=True)
            nc.scalar.activation(B_sb[:, sl], pb[:], AF.Copy)
        for t in range(NT):
            eng = nc.vector if t % 2 == 0 else nc.gpsimd
            n1 = tmp.tile([128, BC], f32, tag="tm", name="n1")
            eng.tensor_tensor(n1[:], xln[t][:].bitcast(f32), A_sb[:], ALU.mult)
            eng.tensor_tensor(n1[:], n1[:], B_sb[:], ALU.add)
            eng.tensor_scalar(
                out=n1[:], in0=n1[:],
                scalar1=bias_sb[:, 19, t:t + 1], scalar2=bias_sb[:, 20, t:t + 1],
                op0=ALU.mult, op1=ALU.add)
            nc.sync.dma_start(out[t * 128:(t + 1) * 128, :], n1[:])

        for p in reversed(ctxs):
            p.__exit__(None, None, None)

    nc.compile()
    return nc


def _prep_host(inputs):
    """Transpose/pack host inputs (layout only, no math)."""
    views = np.asarray(inputs["views"], np.float32)

    def t3(a):
        return np.ascontiguousarray(np.asarray(a, np.float32).transpose(0, 2, 1))

    w = {
        "wq": t3(inputs["Wq"]), "wk": t3(inputs["Wk"]), "wv": t3(inputs["Wv"]),
        "wiq": t3(inputs["Wiq"]), "wik": t3(inputs["Wik"]),
        "wiv": t3(inputs["Wiv"]), "wo": t3(inputs["Wo"]),
        "wout": np.ascontiguousarray(
            np.asarray(inputs["Wout"], np.float32).T.reshape(V, H, H)),
    }

    def bcol(vec):
        return np.asarray(vec, np.float32).reshape(NT, 128).T

    bp = np.zeros((21, 128, NT), np.float32)
    for v in range(V):
        bp[0 + v] = bcol(inputs["bq"][v])
        bp[3 + v] = bcol(inputs["bk"][v])
        bp[6 + v] = bcol(inputs["bv"][v])
        bp[9 + v] = bcol(inputs["biq"][v])
        bp[12 + v] = bcol(inputs["biv"][v])
        bp[15 + v] = bcol(inputs["bo"][v])
    bp[18] = bcol(inputs["bout"])
    bp[19] = bcol(inputs["gamma"])
    bp[20] = bcol(inputs["beta"])
    w["bpk"] = bp
    w["onesc"] = np.ones((128, 128), np.float32)

    xts = []
    for c in range(N_CORES):
        sl = views[:, c * BC:(c + 1) * BC, :]
        xts.append(np.ascontiguousarray(sl.transpose(0, 2, 1)))
    return w, xts


def kernel(**inputs):
    from concourse.bass_utils import run_bass_kernel_spmd

    trace = bool(_CACHE.get("trace", False))
    if "nc" not in _CACHE:
        _CACHE["nc"] = _build_program()
    nc = _CACHE["nc"]

    w, xts = _prep_host(inputs)
    in_maps = []
    for c in range(N_CORES):
        m = dict(w)
        m["xT"] = xts[c]
        in_maps.append(m)

    res = run_bass_kernel_spmd(nc, in_maps, core_ids=list(range(N_CORES)),
                               trace=trace)
    _CACHE["last_result"] = res

    outp = np.empty((B, H), np.float32)
    for c in range(N_CORES):
        outp[c * BC:(c + 1) * BC, :] = res.results[c]["out"].T
    return outp
